# revision 16
# baseline (speedup 1.0000x reference)
"""BASE-layer MoE kernel for Trainium2, expert-parallel across 8 NeuronCores.

Strategy (matches the expert-parallel sharding hint):
  - Routing/balanced assignment is replicated (computed once with the exact
    same jax ops as the reference so the permutation matches bit-for-bit),
    tokens are permuted into [E, C, D] on the host, and each of the 8 cores
    runs its own expert's 2-layer residual FFN (LN -> ff1 -> relu -> ff2 ->
    residual, then sigmoid-gated by the token/centroid affinity).
  - ln_gamma/ln_beta are folded into W1/b1 on the host (exact algebra):
      W1_eff = W1 * gamma[None, :],  b1_eff = b1 + W1 @ beta
  - Matmuls run in bf16 (fp32 accumulation in PSUM); LN statistics, the
    residual stream and the alpha gate stay fp32.

v4 pipeline layout (504.9us baseline):
  - Head: first matmul needs only x[ct0..3] + w1 chunk 0; everything
    (x, w1, w2) rides the ONE sync HWDGE ring whose FIFO order doubles as a
    priority order — w1f0, x0..3 (bf16, half the bytes), w1f1..15, x4..7,
    then w2 chunks interleaved into ff1's chunk stream. Small constants go
    on the gpsimd (SWDGE) ring. ~3.4us of dummy matmuls warm the PE clock
    gate (HAM) and dummy activations preload the 4 ACT function tables
    during the DMA dead time.
  - The residual stream xs is bf16 (rel_err 0.25% -> 0.39%, gate is 2e-2):
    halves the head-critical x bytes and doubles DVE add/gate throughput.
    The final add+gate goes through an fp32 tile for the store.
  - Layer boundary: ff2(h=1)'s DVE eviction adds are deferred in emission
    order past the next layer's LN/transpose/copy chain so the PE's
    transpose feed isn't stuck behind 1.1us adds in the DVE FIFO.
"""

import numpy as np

import concourse.bass as bass
import concourse.mybir as mybir
import concourse.tile as tile
from concourse.masks import make_identity
from concourse.bass_utils import run_bass_kernel_spmd

S, B, D, F, E, L = 2048, 4, 1024, 4096, 8, 2
EPS = 1e-5
T = S * B
C = T // E
P = 128
DT = D // P   # 8 d tiles
FT = F // P   # 32 f tiles
CT = C // P   # 8 c tiles
CH = CT // 2  # 4 c tiles per half
F32 = mybir.dt.float32
BF16 = mybir.dt.bfloat16

# ---------------------------------------------------------------------------
# Workaround: this walrus build rejects >1 sync wait on one instruction
# ("Too many sync wait commands"), but Tile routinely attaches several. After
# tracing, split excess waits onto same-engine NOPs inserted just before the
# instruction — the engine stalls at the NOPs instead, semantics unchanged.
# ---------------------------------------------------------------------------
_MAX_WAITS = 1


def _split_multi_waits(nc, limit=_MAX_WAITS):
    n_split = 0
    for f in nc.m.functions:
        for bb in f.blocks:
            insts = bb.instructions
            out = []
            changed = False
            for ins in insts:
                si = getattr(ins, "sync_info", None)
                if si is not None and si.on_wait and len(si.on_wait) > limit:
                    waits = list(si.on_wait)
                    head, tail = waits[:-limit], waits[-limit:]
                    for i in range(0, len(head), limit):
                        n_split += 1
                        nop = mybir.InstNoOp(
                            name=f"waitsplit-{n_split}",
                            engine=ins.engine,
                            text_hint="waitsplit",
                            bass_nofuse=True,
                        )
                        nop.sync_info = mybir.SyncInfo(
                            on_wait=head[i : i + limit], on_update=[]
                        )
                        out.append(nop)
                    ins.sync_info = mybir.SyncInfo(
                        on_wait=tail, on_update=list(si.on_update or [])
                    )
                    changed = True
                out.append(ins)
            if changed:
                bb.instructions = out
    return n_split


# ---------------------------------------------------------------------------
# Device program (identical on all 8 cores; per-core data differs)
# ---------------------------------------------------------------------------
def _bcast_ap(ap, parts=P):
    """Partition-stride-0 broadcast of a 1-D DRAM AP to [parts, n]."""
    return bass.AP(tensor=ap.tensor, offset=ap.offset, ap=[[0, parts], *ap.ap])


def build_bass(split_waits=True):
    nc = bass.Bass()
    x_d = nc.declare_dram_parameter("x", [C, D], BF16, isOutput=False)
    w1_d = nc.declare_dram_parameter("w1", [L, FT, P, DT, P], BF16, isOutput=False)
    b1_d = nc.declare_dram_parameter("b1", [L, P, FT], F32, isOutput=False)
    w2_d = nc.declare_dram_parameter("w2", [L, P, FT, D], BF16, isOutput=False)
    b2_d = nc.declare_dram_parameter("b2", [L, D], F32, isOutput=False)
    cen_d = nc.declare_dram_parameter("cen", [D], F32, isOutput=False)
    y_d = nc.declare_dram_parameter("y", [C, D], F32, isOutput=True)

    with tile.TileContext(nc) as tc:
        import contextlib

        with contextlib.ExitStack() as ctx:
            singles = ctx.enter_context(tc.tile_pool(name="singles", bufs=1))
            xpool = ctx.enter_context(tc.tile_pool(name="xpool", bufs=1))
            htpool = ctx.enter_context(tc.tile_pool(name="htpool", bufs=1))
            h1pool = ctx.enter_context(tc.tile_pool(name="h1pool", bufs=1))
            w2pool = ctx.enter_context(tc.tile_pool(name="w2pool", bufs=1))
            w1pool = ctx.enter_context(tc.tile_pool(name="w1pool", bufs=16))
            tmps = ctx.enter_context(tc.tile_pool(name="tmps", bufs=3))
            stats = ctx.enter_context(tc.tile_pool(name="stats", bufs=6))
            ps1 = ctx.enter_context(tc.tile_pool(name="ps1", bufs=3, space="PSUM"))
            ps2 = ctx.enter_context(tc.tile_pool(name="ps2", bufs=3, space="PSUM"))
            pst = ctx.enter_context(tc.tile_pool(name="pst", bufs=2, space="PSUM"))

            # --- head DMA order on the sync ring: the first ff1 matmul needs
            # w1 chunk 0 + x[ct0..3]; everything else comes later.
            w1_pre = []
            w1c0 = w1pool.tile([P, DT, P], BF16, tag="w1c")
            nc.sync.dma_start(out=w1c0, in_=w1_d[0, 0])
            w1_pre.append(w1c0)
            xs = []
            for ct in range(CT):
                xt = xpool.tile([P, D], BF16, tag=f"x{ct}")
                xs.append(xt)
            for ct in range(CH):
                nc.sync.dma_start(
                    out=xs[ct], in_=x_d[ct * P : (ct + 1) * P, :]
                )
            for ft in range(1, 16):
                w1c = w1pool.tile([P, DT, P], BF16, tag="w1c")
                nc.sync.dma_start(out=w1c, in_=w1_d[0, ft])
                w1_pre.append(w1c)
            b1_sb = singles.tile([P, L, FT], F32)
            for l in range(L):
                nc.sync.dma_start(out=b1_sb[:, l, :], in_=b1_d[l])
            for ct in range(CH, CT):
                nc.sync.dma_start(
                    out=xs[ct], in_=x_d[ct * P : (ct + 1) * P, :]
                )

            # --- constants: small loads on the gpsimd (SWDGE) ring so they
            # never queue in front of x / w1 on the sync ring.
            eps_t = singles.tile([P, 1], F32)
            nc.vector.memset(eps_t, EPS)
            ident = singles.tile([P, P], BF16)
            make_identity(nc, ident)

            # HAM warm-up: ~3.4us of dummy matmuls while the head DMAs land.
            # The PE clock gate starts at 1.2GHz and only releases to 2.4GHz
            # after ~3.4us of sustained activity; burning that window on
            # throwaway work means the real matmul stream runs warm from its
            # first instruction (saves ~10us of half-clock matmuls).
            warm = ps1.tile([P, 512], F32, tag="ps1")
            for _ in range(32):
                nc.tensor.matmul(
                    warm[:, :P], lhsT=ident, rhs=ident, start=True, stop=True
                )
            # Preload the ACT function tables (Identity/Relu/Sqrt/Sigmoid)
            # during the head DMA dead time: the first use of each function
            # otherwise costs a ~1.3us ACT_TABLE_LOAD right in the middle of
            # the relu-evict stream.
            tl = stats.tile([P, 1], F32, tag="tbl")
            for fn in (
                mybir.ActivationFunctionType.Identity,
                mybir.ActivationFunctionType.Relu,
                mybir.ActivationFunctionType.Sqrt,
                mybir.ActivationFunctionType.Sigmoid,
            ):
                nc.scalar.activation(out=tl, in_=eps_t, func=fn)
            # Gate the constant broadcasts behind x[ct3]: their SDMA packets
            # otherwise steal ~half the fabric from the head-critical x tiles
            # (x landings were measured 3us apart instead of 0.7us).
            dly = stats.tile([P, 1], BF16, tag="dly")
            nc.gpsimd.tensor_copy(out=dly, in_=xs[CH - 1][:, 0:1])
            cen_b = singles.tile([P, D], F32)
            nc.gpsimd.dma_start(out=cen_b, in_=_bcast_ap(cen_d[:]))
            alpha = singles.tile([P, CT], F32)
            b2_b = singles.tile([P, L, D], F32)
            for l in range(L):
                nc.gpsimd.dma_start(out=b2_b[:, l, :], in_=_bcast_ap(b2_d[l]))

            # ht split per c-half so ff1 of half 0 only depends on the first
            # 4 token tiles' LN/transpose (whole-tile dep tracking)
            ht_h = [
                htpool.tile([P, DT, C // 2], BF16, tag=f"ht{h}", name=f"ht{h}")
                for h in range(2)
            ]
            # h1 holds one c-half at a time (ff2 of a half runs before ff1 of
            # the next); split by ft-half so ff2's first 16 ft matmuls only
            # depend on the first 16 relu evictions (no whole-tile bubble).
            h1_f = [
                h1pool.tile([P, FT // 2, C // 2], BF16, tag=f"h1{f}", name=f"h1{f}")
                for f in range(2)
            ]
            w2_sb = w2pool.tile([P, FT, D], BF16)

            def emit_ln(l, ct):
                """LayerNorm of x[ct] (token-major) into h_tm, then transpose
                the 8 [128,128] blocks into ht. Stats on DVE, apply on ACT."""
                st = stats.tile([P, 2, 6], F32, tag="bn_st")
                xin = xs[ct].rearrange("p (s q) -> p s q", s=2)
                for s in range(2):
                    nc.vector.bn_stats(out=st[:, s, :], in_=xin[:, s, :])
                mv = stats.tile([P, 2], F32, tag="bn_mv")
                nc.vector.bn_aggr(out=mv, in_=st)
                nc.scalar.activation(
                    out=mv[:, 1:2],
                    in_=mv[:, 1:2],
                    func=mybir.ActivationFunctionType.Sqrt,
                    bias=eps_t,
                    scale=1.0,
                )
                nc.vector.reciprocal(out=mv[:, 1:2], in_=mv[:, 1:2])
                nb = stats.tile([P, 1], F32, tag="negmr")
                nc.vector.tensor_scalar(
                    out=nb,
                    in0=mv[:, 0:1],
                    scalar1=mv[:, 1:2],
                    scalar2=-1.0,
                    op0=mybir.AluOpType.mult,
                    op1=mybir.AluOpType.mult,
                )
                h_tm = tmps.tile([P, D], BF16, tag="h_tm")
                nc.scalar.activation(
                    out=h_tm,
                    in_=xs[ct],
                    func=mybir.ActivationFunctionType.Identity,
                    bias=nb,
                    scale=mv[:, 1:2],
                )
                hh = ct // CH
                cl = ct % CH
                for dt in range(DT):
                    tp = pst.tile([P, P], BF16, tag="tpsum")
                    nc.tensor.transpose(
                        tp, h_tm[:, dt * P : (dt + 1) * P], ident
                    )
                    nc.vector.tensor_copy(
                        out=ht_h[hh][:, dt, cl * P : (cl + 1) * P], in_=tp
                    )

            def emit_alpha(ct):
                # alpha = sigmoid(x0 . centroid)
                # (tensor_tensor_reduce would fuse these, but this walrus
                # build can't encode it — "ISA wrong length")
                junk = tmps.tile([P, D], F32, tag="alpha_junk")
                dot = stats.tile([P, 1], F32, tag="alpha_dot")
                nc.vector.tensor_mul(out=junk, in0=xs[ct], in1=cen_b)
                nc.vector.reduce_sum(out=dot, in_=junk, axis=mybir.AxisListType.X)
                nc.scalar.activation(
                    out=alpha[:, ct : ct + 1],
                    in_=dot,
                    func=mybir.ActivationFunctionType.Sigmoid,
                )

            def emit_ff1(l, h):
                """h1[f, c-half] = relu(W1eff^T.T @ ht + b1)"""
                csl = slice(h * (C // 2), (h + 1) * (C // 2))
                for ft in range(FT):
                    if l == 0 and h == 0 and ft < len(w1_pre):
                        w1c = w1_pre[ft]
                    else:
                        w1c = w1pool.tile([P, DT, P], BF16, tag="w1c")
                        nc.sync.dma_start(out=w1c, in_=w1_d[l, ft])
                    pt = ps1.tile([P, 512], F32, tag="ps1")
                    for dt in range(DT):
                        nc.tensor.matmul(
                            pt,
                            lhsT=w1c[:, dt, :],
                            rhs=ht_h[h][:, dt, :],
                            start=(dt == 0),
                            stop=(dt == DT - 1),
                        )
                    nc.scalar.activation(
                        out=h1_f[ft // (FT // 2)][:, ft % (FT // 2), :],
                        in_=pt,
                        func=mybir.ActivationFunctionType.Relu,
                        bias=b1_sb[:, l, ft : ft + 1],
                        scale=1.0,
                    )
                    if h == 0 and ft % 8 == 7:
                        # stream this layer's W2 on the SAME sync ring so the
                        # ring's FIFO order gives the head-critical x/w1
                        # transfers strict priority (a parallel-ring w2 load
                        # halves their bandwidth and stalls the head ~20us)
                        k = ft // 8
                        nc.sync.dma_start(
                            out=w2_sb[:, 8 * k : 8 * (k + 1), :],
                            in_=w2_d[l][:, 8 * k : 8 * (k + 1), :],
                        )


            pending_evicts = []

            def flush_evicts():
                for ev in pending_evicts:
                    ev()
                pending_evicts.clear()

            def emit_ff2(l, h, defer=False):
                """x[ct] += W2 @ h1 (+b2 pre-added); last layer: gate+store.
                With defer=True the DVE eviction adds are emitted later
                (after the next layer's LN) so the LN->transpose->copy chain
                feeding the PE isn't stuck behind 1.1us residual adds in the
                DVE FIFO at the layer boundary."""
                for ctl in range(CH):
                    ct = h * CH + ctl
                    # very last token tile: fold the residual into PSUM with
                    # one extra identity matmul so the eviction is a single
                    # ACT op (scale=alpha) + store — keeps the kernel tail
                    # off the DVE FIFO entirely
                    pe_resid = l == L - 1 and ct == CT - 1
                    for dh in range(2):
                        dsl = slice(dh * 512, (dh + 1) * 512)
                        pt2 = ps2.tile([P, 512], F32, tag="ps2")
                        for ft in range(FT):
                            nc.tensor.matmul(
                                pt2,
                                lhsT=h1_f[ft // (FT // 2)][
                                    :, ft % (FT // 2), ctl * P : (ctl + 1) * P
                                ],
                                rhs=w2_sb[:, ft, dsl],
                                start=(ft == 0),
                                stop=(ft == FT - 1) and not pe_resid,
                            )
                        if pe_resid:
                            nc.tensor.matmul(
                                pt2,
                                lhsT=ident,
                                rhs=xs[ct][:, dsl],
                                start=False,
                                stop=True,
                            )
                            yt = tmps.tile([P, 512], F32, tag="yt")
                            nc.scalar.activation(
                                out=yt,
                                in_=pt2,
                                func=mybir.ActivationFunctionType.Identity,
                                scale=alpha[:, ct : ct + 1],
                            )
                            nc.scalar.dma_start(
                                out=y_d[ct * P : (ct + 1) * P, dsl], in_=yt
                            )
                            continue
                        if l < L - 1:

                            def ev(ct=ct, dsl=dsl, pt2=pt2):
                                nc.vector.tensor_add(
                                    out=xs[ct][:, dsl],
                                    in0=xs[ct][:, dsl],
                                    in1=pt2,
                                )

                            # Defer ONLY the final c-tile's evictions: they
                            # are the ones whose DVE adds collide with the
                            # next layer's transpose-copy chain, and nothing
                            # waits on their psum slots (deferring earlier
                            # groups would deadlock: their slots gate later
                            # ff2 matmuls, while the deferred adds sit behind
                            # copies that wait on transposes queued after
                            # those same matmuls).
                            if defer and ctl >= CH - 2:
                                pending_evicts.append(ev)
                            else:
                                ev()
                        else:
                            # final layer: add + gate into an fp32 tile (the
                            # bf16 residual can't be DMA'd to the fp32 output
                            # directly), per d-half so the last token tile's
                            # store chain is short
                            yt = tmps.tile([P, 512], F32, tag="yt")
                            nc.vector.tensor_add(
                                out=yt, in0=xs[ct][:, dsl], in1=pt2
                            )
                            nc.vector.tensor_scalar_mul(
                                out=yt, in0=yt, scalar1=alpha[:, ct : ct + 1]
                            )
                            nc.scalar.dma_start(
                                out=y_d[ct * P : (ct + 1) * P, dsl], in_=yt
                            )

            for l in range(L):
                for ct in range(CH):
                    emit_ln(l, ct)
                flush_evicts()
                emit_ff1(l, 0)
                # DVE extras emitted after ff1 so they can't delay the
                # LN->transpose->copy chain feeding the PE; alpha must read
                # xs[ct] before the b2 pre-add writes it (DVE FIFO order)
                for ct in range(CH):
                    if l == 0:
                        emit_alpha(ct)
                    nc.vector.tensor_add(
                        out=xs[ct], in0=xs[ct], in1=b2_b[:, l, :]
                    )
                for ct in range(CH, CT):
                    emit_ln(l, ct)
                emit_ff2(l, 0)
                for ct in range(CH, CT):
                    if l == 0:
                        emit_alpha(ct)
                    nc.vector.tensor_add(
                        out=xs[ct], in0=xs[ct], in1=b2_b[:, l, :]
                    )
                emit_ff1(l, 1)
                emit_ff2(l, 1, defer=(l < L - 1))
    if split_waits:
        _split_multi_waits(nc)
    return nc


_NC_CACHE = None


def _get_nc():
    global _NC_CACHE
    if _NC_CACHE is None:
        _NC_CACHE = build_bass()
    return _NC_CACHE


# ---------------------------------------------------------------------------
# Host side: routing (replicated, bit-exact with the reference) + sharding
# ---------------------------------------------------------------------------
def _routing_perm(features, centroids):
    # Replicates the reference's _balanced_assignment with the exact same jax
    # ops, pinned to the CPU backend: the reference itself can only run on
    # CPU jax (stable sort doesn't compile for the neuron backend), so CPU
    # numerics are the ones the permutation must match bit-for-bit.
    import jax
    import jax.numpy as jnp

    with jax.default_device(jax.devices("cpu")[0]):
        feats = jnp.asarray(features)
        cents = jnp.asarray(centroids)
        aff = jax.lax.stop_gradient(feats) @ jax.lax.stop_gradient(cents).T
        aff = jnp.nan_to_num(aff)
        capacity = feats.shape[0] // cents.shape[0]
        order = jnp.argsort(-aff.max(axis=1))
        aff_ord = aff[order]

        def step(counts, row):
            masked = jnp.where(counts < capacity, row, -jnp.inf)
            e = jnp.argmax(masked).astype(jnp.int32)
            return counts.at[e].add(1), e

        _, assign_ord = jax.lax.scan(
            step, jnp.zeros(cents.shape[0], jnp.int32), aff_ord
        )
        assign = jnp.zeros(feats.shape[0], jnp.int32).at[order].set(assign_ord)
        return np.asarray(jnp.argsort(assign))


def _prep_core_inputs(xr, centroids, ln_gamma, ln_beta, W1, b1, W2, b2):
    """Per-core input maps; folds gamma/beta into W1/b1 and pre-tiles weights."""
    maps = []
    for e in range(E):
        m = {"x": None}
        w1s = np.empty((L, FT, P, DT, P), np.float32)
        w2s = np.empty((L, P, FT, D), np.float32)
        b1s = np.empty((L, P, FT), np.float32)
        for l in range(L):
            g = ln_gamma[l, e]
            bt = ln_beta[l, e]
            w1_eff = W1[l, e] * g[None, :]          # [F, D]
            b1_eff = b1[l, e] + W1[l, e] @ bt       # [F]
            # lhsT tiles: w1s[l, ft, p_d, dt, j_f] = w1_eff[ft*P+j, dt*P+p]
            w1s[l] = w1_eff.reshape(FT, P, DT, P).transpose(0, 3, 2, 1)
            # w2s[l, p_f, ft, d] = W2[l,e][d, ft*P+p]
            w2s[l] = W2[l, e].T.reshape(FT, P, D).transpose(1, 0, 2)
            b1s[l] = b1_eff.reshape(FT, P).T
        import ml_dtypes

        m["x"] = np.ascontiguousarray(xr[e]).astype(ml_dtypes.bfloat16)
        m["w1"] = w1s.astype(ml_dtypes.bfloat16)
        m["w2"] = w2s.astype(ml_dtypes.bfloat16)
        m["b1"] = b1s
        m["b2"] = np.ascontiguousarray(b2[:, e, :]).astype(np.float32)
        m["cen"] = np.ascontiguousarray(centroids[e]).astype(np.float32)
        maps.append(m)
    return maps


def kernel(
    input_features,
    centroids,
    ln_gamma,
    ln_beta,
    W1,
    b1,
    W2,
    b2,
    input_ids=None,
    _trace=False,
    _tmpdir=None,
):
    input_features = np.asarray(input_features, np.float32)
    centroids = np.asarray(centroids, np.float32)
    ln_gamma = np.asarray(ln_gamma, np.float32)
    ln_beta = np.asarray(ln_beta, np.float32)
    W1 = np.asarray(W1, np.float32)
    b1 = np.asarray(b1, np.float32)
    W2 = np.asarray(W2, np.float32)
    b2 = np.asarray(b2, np.float32)

    feats = input_features.reshape(T, D)
    perm = _routing_perm(feats, centroids)
    xr = feats[perm].reshape(E, C, D)

    maps = _prep_core_inputs(xr, centroids, ln_gamma, ln_beta, W1, b1, W2, b2)
    nc = _get_nc()
    res = run_bass_kernel_spmd(
        nc, maps, list(range(E)), trace=_trace, tmpdir=_tmpdir
    )
    y = np.concatenate([res.results[e]["y"] for e in range(E)], axis=0)  # [T, D]
    out = np.zeros((T, D), np.float32)
    out[perm] = y
    out = out.reshape(input_features.shape)
    if _trace:
        return out, res
    return out


# revision 17
# speedup vs baseline: 1.0034x; 1.0034x over previous
"""BASE-layer MoE kernel for Trainium2, expert-parallel across 8 NeuronCores.

Strategy (matches the expert-parallel sharding hint):
  - Routing/balanced assignment is replicated (computed once with the exact
    same jax ops as the reference so the permutation matches bit-for-bit),
    tokens are permuted into [E, C, D] on the host, and each of the 8 cores
    runs its own expert's 2-layer residual FFN (LN -> ff1 -> relu -> ff2 ->
    residual, then sigmoid-gated by the token/centroid affinity).
  - ln_gamma/ln_beta are folded into W1/b1 on the host (exact algebra):
      W1_eff = W1 * gamma[None, :],  b1_eff = b1 + W1 @ beta
  - Matmuls run in bf16 (fp32 accumulation in PSUM); LN statistics, the
    residual stream and the alpha gate stay fp32.

Perf notes (measured on HW, 8 cores, max-core exec time):
  - This version: ~504.7-504.9us. PE matmul track is 89.8% occupied
    (459us busy of 505; steady-state windows are 99.6% dense). The bf16
    matmul roofline for the 4x 1024x1024x4096 GEMMs per core is 437us, plus
    ~7us of PE transposes and ~2ns/MM issue overhead -> ~448us stream floor.
  - The remaining ~46us is head/tail DMA completion latency (~6us
    issue-to-semaphore for the first x tile, ~2-3us spacing thereafter,
    independent of ring ordering), a ~7us framework preamble inside the
    measured window, and ~5us layer-boundary DVE interleave.
  - Variants tried and measured SLOWER or equal (so not kept): reordered
    per-half LN/ff1 emission + HAM warm-up matmuls + ACT-table preloads +
    bf16 residual stream (506.7us); plus deferred ff2 evictions, gated
    constant broadcasts, PE-residual tail (507.6us); w2 on the gpsimd ring
    (535us - fabric competition with head-critical x/w1 loads).
  - Dead ends proven out: fp8-e4m3 DoubleRow (2x PE rate) fails the 2e-2
    gate - measured 4.0% rel err all-fp8, ~2.9% with any single fp8 tensor
    (vs 0.25% bf16); int8/uint8 matmul is not supported by the trn2
    (Cayman) compiler target; 1024-wide moving operands fail walrus
    codegen ("ISA wrong length" - PSUM dst can't span banks); DVE
    tensor_tensor_reduce is unencodable on this walrus build.
"""

import numpy as np

import concourse.bass as bass
import concourse.mybir as mybir
import concourse.tile as tile
from concourse.masks import make_identity
from concourse.bass_utils import run_bass_kernel_spmd

S, B, D, F, E, L = 2048, 4, 1024, 4096, 8, 2
EPS = 1e-5
T = S * B
C = T // E
P = 128
DT = D // P   # 8 d tiles
FT = F // P   # 32 f tiles
CT = C // P   # 8 c tiles
F32 = mybir.dt.float32
BF16 = mybir.dt.bfloat16

# ---------------------------------------------------------------------------
# Workaround: this walrus build rejects >1 sync wait on one instruction
# ("Too many sync wait commands"), but Tile routinely attaches several. After
# tracing, split excess waits onto same-engine NOPs inserted just before the
# instruction — the engine stalls at the NOPs instead, semantics unchanged.
# ---------------------------------------------------------------------------
_MAX_WAITS = 1


def _split_multi_waits(nc, limit=_MAX_WAITS):
    n_split = 0
    for f in nc.m.functions:
        for bb in f.blocks:
            insts = bb.instructions
            out = []
            changed = False
            for ins in insts:
                si = getattr(ins, "sync_info", None)
                if si is not None and si.on_wait and len(si.on_wait) > limit:
                    waits = list(si.on_wait)
                    head, tail = waits[:-limit], waits[-limit:]
                    for i in range(0, len(head), limit):
                        n_split += 1
                        nop = mybir.InstNoOp(
                            name=f"waitsplit-{n_split}",
                            engine=ins.engine,
                            text_hint="waitsplit",
                            bass_nofuse=True,
                        )
                        nop.sync_info = mybir.SyncInfo(
                            on_wait=head[i : i + limit], on_update=[]
                        )
                        out.append(nop)
                    ins.sync_info = mybir.SyncInfo(
                        on_wait=tail, on_update=list(si.on_update or [])
                    )
                    changed = True
                out.append(ins)
            if changed:
                bb.instructions = out
    return n_split


# ---------------------------------------------------------------------------
# Device program (identical on all 8 cores; per-core data differs)
# ---------------------------------------------------------------------------
def _bcast_ap(ap, parts=P):
    """Partition-stride-0 broadcast of a 1-D DRAM AP to [parts, n]."""
    return bass.AP(tensor=ap.tensor, offset=ap.offset, ap=[[0, parts], *ap.ap])


def build_bass(split_waits=True):
    nc = bass.Bass()
    x_d = nc.declare_dram_parameter("x", [C, D], F32, isOutput=False)
    w1_d = nc.declare_dram_parameter("w1", [L, FT, P, DT, P], BF16, isOutput=False)
    b1_d = nc.declare_dram_parameter("b1", [L, P, FT], F32, isOutput=False)
    w2_d = nc.declare_dram_parameter("w2", [L, P, FT, D], BF16, isOutput=False)
    b2_d = nc.declare_dram_parameter("b2", [L, D], F32, isOutput=False)
    cen_d = nc.declare_dram_parameter("cen", [D], F32, isOutput=False)
    y_d = nc.declare_dram_parameter("y", [C, D], F32, isOutput=True)

    with tile.TileContext(nc) as tc:
        import contextlib

        with contextlib.ExitStack() as ctx:
            singles = ctx.enter_context(tc.tile_pool(name="singles", bufs=1))
            xpool = ctx.enter_context(tc.tile_pool(name="xpool", bufs=1))
            htpool = ctx.enter_context(tc.tile_pool(name="htpool", bufs=1))
            h1pool = ctx.enter_context(tc.tile_pool(name="h1pool", bufs=1))
            w2pool = ctx.enter_context(tc.tile_pool(name="w2pool", bufs=1))
            w1pool = ctx.enter_context(tc.tile_pool(name="w1pool", bufs=16))
            tmps = ctx.enter_context(tc.tile_pool(name="tmps", bufs=3))
            stats = ctx.enter_context(tc.tile_pool(name="stats", bufs=6))
            ps1 = ctx.enter_context(tc.tile_pool(name="ps1", bufs=3, space="PSUM"))
            ps2 = ctx.enter_context(tc.tile_pool(name="ps2", bufs=3, space="PSUM"))
            pst = ctx.enter_context(tc.tile_pool(name="pst", bufs=2, space="PSUM"))

            # --- x loads first: they gate the LN -> transpose -> ff1 chain.
            # Split per ct and per d-half so per-queue latency stays low.
            xs = []
            for ct in range(CT):
                xt = xpool.tile([P, D], F32, tag=f"x{ct}")
                for dh in range(2):
                    nc.sync.dma_start(
                        out=xt[:, dh * 512 : (dh + 1) * 512],
                        in_=x_d[ct * P : (ct + 1) * P, dh * 512 : (dh + 1) * 512],
                    )
                xs.append(xt)

            # --- constants (small; issued on gpsimd to stay off the SP path)
            eps_t = singles.tile([P, 1], F32)
            nc.vector.memset(eps_t, EPS)
            ident = singles.tile([P, P], BF16)
            make_identity(nc, ident)
            cen_b = singles.tile([P, D], F32)
            nc.gpsimd.dma_start(out=cen_b, in_=_bcast_ap(cen_d[:]))
            alpha = singles.tile([P, CT], F32)
            b1_sb = singles.tile([P, L, FT], F32)
            for l in range(L):
                nc.sync.dma_start(out=b1_sb[:, l, :], in_=b1_d[l])
            b2_b = singles.tile([P, L, D], F32)
            for l in range(L):
                nc.gpsimd.dma_start(out=b2_b[:, l, :], in_=_bcast_ap(b2_d[l]))

            # ht split per c-half so ff1-ch0 only depends on the first
            # 4 token tiles' LN/transpose (whole-tile dep tracking)
            ht_h = [
                htpool.tile([P, DT, C // 2], BF16, tag=f"ht{h}", name=f"ht{h}")
                for h in range(2)
            ]
            h1 = h1pool.tile([P, FT, C // 2], BF16)  # per c-half: [f_p, ft, c]
            w2_sb = w2pool.tile([P, FT, D], BF16)

            # Pre-issue the first w1 chunks (= pool depth, so no slot waits)
            # and layer-0's w2 ahead of the LN/transpose section: SP executes
            # its DMA stream in order, and a dependent transpose ahead of
            # these would head-of-line-block them for tens of us.
            w1_pre = []
            for ft in range(12):
                w1c = w1pool.tile([P, DT, P], BF16, tag="w1c")
                nc.sync.dma_start(out=w1c, in_=w1_d[0, ft])
                w1_pre.append(w1c)

            def emit_ln(l, ct):
                """LayerNorm of x[ct] (token-major) into h_tm, then transpose
                the 8 [128,128] blocks into ht. Stats on DVE, apply on ACT."""
                st = stats.tile([P, 2, 6], F32, tag="bn_st")
                xin = xs[ct].rearrange("p (s q) -> p s q", s=2)
                for s in range(2):
                    nc.vector.bn_stats(out=st[:, s, :], in_=xin[:, s, :])
                mv = stats.tile([P, 2], F32, tag="bn_mv")
                nc.vector.bn_aggr(out=mv, in_=st)
                nc.scalar.activation(
                    out=mv[:, 1:2],
                    in_=mv[:, 1:2],
                    func=mybir.ActivationFunctionType.Sqrt,
                    bias=eps_t,
                    scale=1.0,
                )
                nc.vector.reciprocal(out=mv[:, 1:2], in_=mv[:, 1:2])
                nb = stats.tile([P, 1], F32, tag="negmr")
                nc.vector.tensor_scalar(
                    out=nb,
                    in0=mv[:, 0:1],
                    scalar1=mv[:, 1:2],
                    scalar2=-1.0,
                    op0=mybir.AluOpType.mult,
                    op1=mybir.AluOpType.mult,
                )
                h_tm = tmps.tile([P, D], BF16, tag="h_tm")
                nc.scalar.activation(
                    out=h_tm,
                    in_=xs[ct],
                    func=mybir.ActivationFunctionType.Identity,
                    bias=nb,
                    scale=mv[:, 1:2],
                )
                hh = ct // (CT // 2)
                cl = ct % (CT // 2)
                for dt in range(DT):
                    tp = pst.tile([P, P], BF16, tag="tpsum")
                    nc.tensor.transpose(
                        tp, h_tm[:, dt * P : (dt + 1) * P], ident
                    )
                    nc.vector.tensor_copy(
                        out=ht_h[hh][:, dt, cl * P : (cl + 1) * P], in_=tp
                    )

            for l in range(L):
                for ct in range(CT):
                    emit_ln(l, ct)
                if l == 0:
                    # alpha = sigmoid(x0 . centroid); emitted after the whole
                    # LN pass so its DVE work doesn't delay the ht transposes
                    for ct in range(CT):
                        junk = tmps.tile([P, D], F32, tag="alpha_junk")
                        dot = stats.tile([P, 1], F32, tag="alpha_dot")
                        nc.vector.tensor_mul(out=junk, in0=xs[ct], in1=cen_b)
                        nc.vector.reduce_sum(out=dot, in_=junk, axis=mybir.AxisListType.X)
                        nc.scalar.activation(
                            out=alpha[:, ct : ct + 1],
                            in_=dot,
                            func=mybir.ActivationFunctionType.Sigmoid,
                        )

                # x += b2 up front (h already extracted; addition commutes
                # with the ff2 accumulation) so the ff2 evict is a single add
                for ct in range(CT):
                    nc.vector.tensor_add(
                        out=xs[ct], in0=xs[ct], in1=b2_b[:, l, :]
                    )

                for ch in range(2):  # c halves of 512
                    csl = slice(ch * (C // 2), (ch + 1) * (C // 2))
                    # --- ff1: h1[f, c] = relu(W1eff^T.T @ ht + b1) ---
                    for ft in range(FT):
                        if l == 0 and ch == 0 and ft < len(w1_pre):
                            w1c = w1_pre[ft]
                        else:
                            w1c = w1pool.tile([P, DT, P], BF16, tag="w1c")
                            nc.sync.dma_start(out=w1c, in_=w1_d[l, ft])
                        pt = ps1.tile([P, 512], F32, tag="ps1")
                        for dt in range(DT):
                            nc.tensor.matmul(
                                pt,
                                lhsT=w1c[:, dt, :],
                                rhs=ht_h[ch][:, dt, :],
                                start=(dt == 0),
                                stop=(dt == DT - 1),
                            )
                        nc.scalar.activation(
                            out=h1[:, ft, :],
                            in_=pt,
                            func=mybir.ActivationFunctionType.Relu,
                            bias=b1_sb[:, l, ft : ft + 1],
                            scale=1.0,
                        )
                    if ch == 0:
                        # issue the layer's W2 load while PE chews on ff1;
                        # delaying it keeps startup queue bandwidth for x/w1
                        nc.sync.dma_start(out=w2_sb, in_=w2_d[l])
                    # --- ff2 + residual (+ b2) ---
                    for ctl in range(CT // 2):
                        ct = ch * (CT // 2) + ctl
                        for dh in range(2):
                            dsl = slice(dh * 512, (dh + 1) * 512)
                            pt2 = ps2.tile([P, 512], F32, tag="ps2")
                            for ft in range(FT):
                                nc.tensor.matmul(
                                    pt2,
                                    lhsT=h1[:, ft, ctl * P : (ctl + 1) * P],
                                    rhs=w2_sb[:, ft, dsl],
                                    start=(ft == 0),
                                    stop=(ft == FT - 1),
                                )
                            nc.vector.tensor_add(
                                out=xs[ct][:, dsl], in0=xs[ct][:, dsl], in1=pt2
                            )
                        if l == L - 1:
                            # final gate + output as soon as this ct is done
                            nc.vector.tensor_scalar_mul(
                                out=xs[ct],
                                in0=xs[ct],
                                scalar1=alpha[:, ct : ct + 1],
                            )
                            nc.sync.dma_start(
                                out=y_d[ct * P : (ct + 1) * P, :], in_=xs[ct]
                            )
    if split_waits:
        _split_multi_waits(nc)
    return nc


_NC_CACHE = None


def _get_nc():
    global _NC_CACHE
    if _NC_CACHE is None:
        _NC_CACHE = build_bass()
    return _NC_CACHE


# ---------------------------------------------------------------------------
# Host side: routing (replicated, bit-exact with the reference) + sharding
# ---------------------------------------------------------------------------
def _routing_perm(features, centroids):
    # Replicates the reference's _balanced_assignment with the exact same jax
    # ops, pinned to the CPU backend: the reference itself can only run on
    # CPU jax (stable sort doesn't compile for the neuron backend), so CPU
    # numerics are the ones the permutation must match bit-for-bit.
    import jax
    import jax.numpy as jnp

    with jax.default_device(jax.devices("cpu")[0]):
        feats = jnp.asarray(features)
        cents = jnp.asarray(centroids)
        aff = jax.lax.stop_gradient(feats) @ jax.lax.stop_gradient(cents).T
        aff = jnp.nan_to_num(aff)
        capacity = feats.shape[0] // cents.shape[0]
        order = jnp.argsort(-aff.max(axis=1))
        aff_ord = aff[order]

        def step(counts, row):
            masked = jnp.where(counts < capacity, row, -jnp.inf)
            e = jnp.argmax(masked).astype(jnp.int32)
            return counts.at[e].add(1), e

        _, assign_ord = jax.lax.scan(
            step, jnp.zeros(cents.shape[0], jnp.int32), aff_ord
        )
        assign = jnp.zeros(feats.shape[0], jnp.int32).at[order].set(assign_ord)
        return np.asarray(jnp.argsort(assign))


def _prep_core_inputs(xr, centroids, ln_gamma, ln_beta, W1, b1, W2, b2):
    """Per-core input maps; folds gamma/beta into W1/b1 and pre-tiles weights."""
    maps = []
    for e in range(E):
        m = {"x": np.ascontiguousarray(xr[e])}
        w1s = np.empty((L, FT, P, DT, P), np.float32)
        w2s = np.empty((L, P, FT, D), np.float32)
        b1s = np.empty((L, P, FT), np.float32)
        for l in range(L):
            g = ln_gamma[l, e]
            bt = ln_beta[l, e]
            w1_eff = W1[l, e] * g[None, :]          # [F, D]
            b1_eff = b1[l, e] + W1[l, e] @ bt       # [F]
            # lhsT tiles: w1s[l, ft, p_d, dt, j_f] = w1_eff[ft*P+j, dt*P+p]
            w1s[l] = w1_eff.reshape(FT, P, DT, P).transpose(0, 3, 2, 1)
            # w2s[l, p_f, ft, d] = W2[l,e][d, ft*P+p]
            w2s[l] = W2[l, e].T.reshape(FT, P, D).transpose(1, 0, 2)
            b1s[l] = b1_eff.reshape(FT, P).T
        import ml_dtypes

        m["w1"] = w1s.astype(ml_dtypes.bfloat16)
        m["w2"] = w2s.astype(ml_dtypes.bfloat16)
        m["b1"] = b1s
        m["b2"] = np.ascontiguousarray(b2[:, e, :]).astype(np.float32)
        m["cen"] = np.ascontiguousarray(centroids[e]).astype(np.float32)
        maps.append(m)
    return maps


def kernel(
    input_features,
    centroids,
    ln_gamma,
    ln_beta,
    W1,
    b1,
    W2,
    b2,
    input_ids=None,
    _trace=False,
    _tmpdir=None,
):
    input_features = np.asarray(input_features, np.float32)
    centroids = np.asarray(centroids, np.float32)
    ln_gamma = np.asarray(ln_gamma, np.float32)
    ln_beta = np.asarray(ln_beta, np.float32)
    W1 = np.asarray(W1, np.float32)
    b1 = np.asarray(b1, np.float32)
    W2 = np.asarray(W2, np.float32)
    b2 = np.asarray(b2, np.float32)

    feats = input_features.reshape(T, D)
    perm = _routing_perm(feats, centroids)
    xr = feats[perm].reshape(E, C, D)

    maps = _prep_core_inputs(xr, centroids, ln_gamma, ln_beta, W1, b1, W2, b2)
    nc = _get_nc()
    res = run_bass_kernel_spmd(
        nc, maps, list(range(E)), trace=_trace, tmpdir=_tmpdir
    )
    y = np.concatenate([res.results[e]["y"] for e in range(E)], axis=0)  # [T, D]
    out = np.zeros((T, D), np.float32)
    out[perm] = y
    out = out.reshape(input_features.shape)
    if _trace:
        return out, res
    return out
